# revision 1
# baseline (speedup 1.0000x reference)
"""Trainium2 Bass kernel for nn_BreedingPolicyNet (sparse_attention family).

Reference semantics (per wave, 8 waves):
    present_p1 = x > 0;  present_p2 = present_p1 with target_idx forced False
    allowed[a,b] = p1[a] & p2[b]
    Qi = softmax(where(allowed, logits, -FLT_MAX), axis=1), zeroed where row empty
    offspring[k] = sum_{a,b} x[a] * Qi[a,b] * T[a,b,k]
    x = max(x + offspring, 0)

Key algebraic property exploited: when every x0[i] > 0 and T >= 0, x stays
strictly positive through all waves (offspring >= 0), so the mask — and
therefore Qi — is IDENTICAL in every wave.  Then
    S[a,k] = sum_b Qi[a,b] * T[a,b,k]        (one single pass over T)
    offspring = x @ S                         (tiny per-wave matvec)
This turns 8 full 512MB passes over T into one (the memory roofline).

Distribution: shard T along axis a (contiguous 64MB per core).  Each core
computes its 64 rows of S with PE matmuls, an AllGather assembles the full
[512,512] S on every core, and all cores redundantly run the 8-wave
recurrence on-device.  Output is read from core 0.
"""

import numpy as np

N = 512
NC = 8           # NeuronCores
SH = N // NC     # a-rows per core
NWAVES = 8
AB = 8           # a-rows fetched per DMA (8MB chunks)
NEG_LARGE = float(np.finfo(np.float32).min)

_prog_cache = {}
last_results = None  # stash of BassKernelResults for test harness introspection


def _qi_matrix(logits: np.ndarray, tgt: int) -> np.ndarray:
    """Wave-invariant Qi: row softmax of logits with column `tgt` masked."""
    masked = np.array(logits, dtype=np.float32, copy=True)
    masked[:, tgt] = NEG_LARGE
    m = masked.max(axis=1, keepdims=True)
    e = np.exp(masked - m, dtype=np.float32)
    return (e / e.sum(axis=1, keepdims=True, dtype=np.float32)).astype(np.float32)


MM_DTYPE = "f32r"  # "f32" (exact, 4 cyc/row) or "f32r" (TF32-ish, 1 cyc/row)


def _build_program(reps: int = 1, mm: str = MM_DTYPE, nwaves: int = NWAVES,
                   ag: bool = True, ab: int = AB, tbufs: int = 2,
                   sbufs: int = 2, colwise_add: bool = True,
                   dual_ring: bool = False, split_ag: bool = True,
                   taper: bool = True, contig: bool = True):
    """Build + compile the SPMD program.

    reps > 1 emits the whole body N times, serialized end-to-start via an
    explicit dependency and chained through x — used only for benchmarking
    per-execution device time with dispatch overhead amortized out.

    mm selects the S-stage matmul dtype: float32r streams the moving
    operand at full rate (1 cycle/row vs 4 for float32), turning the
    S pass from PE-bound back into DMA-bound.  The wave matmuls always
    stay float32 — their cost is negligible and x spans 19 decades.
    """
    import concourse.bacc as bacc
    import concourse.bass as bass
    import concourse.mybir as mybir
    import concourse.tile as tile

    f32 = mybir.dt.float32
    fmm = mybir.dt.float32r if mm == "f32r" else f32
    nc = bacc.Bacc(
        "TRN2",
        target_bir_lowering=False,
        debug=False,
        enable_asserts=False,
        num_devices=NC,
    )
    t_shard = nc.dram_tensor("t_shard", [SH, N, N], fmm, kind="ExternalInput").ap()
    q_cols = nc.dram_tensor("q_cols", [N, SH], fmm, kind="ExternalInput").ap()
    x0c = nc.dram_tensor("x0c", [128, 4], f32, kind="ExternalInput").ap()
    x_out = nc.dram_tensor("x_out", [128, 4], f32, kind="ExternalOutput").ap()

    with tile.TileContext(nc) as tc:
        with (
            tc.tile_pool(name="const", bufs=1) as cpool,
            tc.tile_pool(name="tbuf", bufs=3) as tpool,
            tc.tile_pool(name="sfull", bufs=1) as spool,
            tc.tile_pool(name="xbuf", bufs=2) as xpool,
            tc.tile_pool(name="psum_s", bufs=6, space="PSUM") as pspool,
            tc.tile_pool(name="psum_w", bufs=2, space="PSUM") as pwpool,
            tc.tile_pool(name="dram", bufs=1, space="DRAM") as dpool,
        ):
            # Qi columns for this core's a-shard: q_cols[b, a] = Qi[a0+a, b]
            qts = []
            for g in range(4):
                qt = cpool.tile([128, SH], fmm, tag=f"qt{g}")
                nc.sync.dma_start(qt[:], q_cols[g * 128:(g + 1) * 128, :])
                qts.append(qt)

            xc = None
            prev_tail = None  # last instruction of previous rep (bench mode)
            if taper:
                sizes = [ab] * (SH // ab - 1) + [ab // 2, ab // 2]
            else:
                sizes = [ab] * (SH // ab)
            assert sum(sizes) == SH
            for rep in range(reps):
                if split_ag:
                    # two half-gathers: the first (rows 0..31) is issued as
                    # soon as those S rows are staged and hides under the
                    # remaining T DMA stream; only the second is a tail.
                    ag_in_h = [dpool.tile([SH // 2, N], f32,
                                          tag=f"ag_in{rep}h{h}",
                                          name=f"ag_in{rep}h{h}")
                               for h in range(2)]
                    ag_out_h = [dpool.tile([NC * (SH // 2), N], f32,
                                           tag=f"ag_out{rep}h{h}",
                                           name=f"ag_out{rep}h{h}")
                               for h in range(2)]
                else:
                    ag_in = dpool.tile([SH, N], f32, tag=f"ag_in{rep}")
                    ag_out = dpool.tile([N, N], f32, tag=f"ag_out{rep}")

                # ---- one pass over the T shard:
                # S[a,:] = sum_b Qi[a,b] * T[a,b,:].  S rows come out of the
                # PE as [1, 512] on partition 0; compute engines can only
                # write 32-aligned partition offsets, so stage SROWS of them
                # side-by-side in the free dim and DMA to DRAM.
                SROWS = 8
                a0 = 0
                for ib, cs in enumerate(sizes):
                    tt = tpool.tile([128, cs * 4 * N], fmm, tag="tt",
                                    bufs=tbufs)
                    src = t_shard[a0:a0 + cs]  # [cs, N, N]
                    eng = nc.scalar if (dual_ring and ib % 2) else nc.sync
                    if contig:
                        # partition p takes the contiguous 8KB span
                        # T[a, 4p:(4p+4), :] -> 4x longer DMA bursts; the
                        # b-contraction regroups as b = 4p+q and the Qi
                        # columns arrive pre-permuted to match (host-side).
                        ld = eng.dma_start(
                            tt[:].rearrange("p (j qk) -> p j qk", j=cs),
                            src.rearrange("j (p q) k -> p j (q k)",
                                          p=128, q=4),
                        )
                    else:
                        ld = eng.dma_start(
                            tt[:].rearrange("p (j g k) -> p j g k",
                                            j=cs, g=4),
                            src.rearrange("j (g p) k -> p j g k", g=4, p=128),
                        )
                    if ib == 0 and prev_tail is not None:
                        bass._add_dep_helper(
                            ld.ins, prev_tail.ins, True, "serialize bench rep")
                    for j in range(cs):
                        a = a0 + j
                        if a % SROWS == 0:
                            stage = cpool.tile([1, SROWS * N], f32,
                                               tag="stage", bufs=sbufs)
                        ps = pspool.tile([1, N], f32, tag="ps")
                        for g in range(4):
                            nc.tensor.matmul(
                                ps[:],
                                lhsT=qts[g][:, a:a + 1],
                                rhs=tt[:, (j * 4 + g) * N:(j * 4 + g + 1) * N],
                                start=(g == 0),
                                stop=(g == 3),
                            )
                        r = a % SROWS
                        nc.vector.tensor_copy(stage[:, r * N:(r + 1) * N],
                                              ps[:])
                        if r == SROWS - 1:
                            if split_ag:
                                dst_t = ag_in_h[(a - r) // (SH // 2)]
                                dst = dst_t[(a - r) % (SH // 2):
                                            (a - r) % (SH // 2) + SROWS, :]
                            else:
                                dst = ag_in[a - r:a + 1, :]
                            nc.sync.dma_start(
                                dst.rearrange("(p r) k -> p r k", p=1),
                                stage[:].rearrange("p (r k) -> p r k",
                                                   r=SROWS),
                            )
                    a0 += cs

                # ---- AllGather the S shards into the full [512, 512] S
                sf = []
                if ag and split_ag:
                    for h in range(2):
                        nc.gpsimd.collective_compute(
                            "AllGather",
                            mybir.AluOpType.bypass,
                            replica_groups=[list(range(NC))],
                            ins=[ag_in_h[h].opt()],
                            outs=[ag_out_h[h].opt()],
                        )
                    HS = SH // 2  # 32
                    for g in range(4):
                        t = spool.tile([128, N], f32, tag=f"sf{g}")
                        # S rows [128g, 128g+128) come from ranks 2g, 2g+1:
                        # out_h[c*32:(c+1)*32] holds S[c*64+h*32 .. +32]
                        for half in range(2):      # rank 2g / 2g+1
                            c = 2 * g + half
                            for h in range(2):     # row half within rank
                                nc.sync.dma_start(
                                    t[half * 64 + h * HS:
                                      half * 64 + (h + 1) * HS, :],
                                    ag_out_h[h][c * HS:(c + 1) * HS, :])
                        sf.append(t)
                elif ag:
                    nc.gpsimd.collective_compute(
                        "AllGather",
                        mybir.AluOpType.bypass,
                        replica_groups=[list(range(NC))],
                        ins=[ag_in.opt()],
                        outs=[ag_out.opt()],
                    )
                    for g in range(4):
                        t = spool.tile([128, N], f32, tag=f"sf{g}")
                        nc.sync.dma_start(t[:], ag_out[g * 128:(g + 1) * 128, :])
                        sf.append(t)
                else:
                    for g in range(4):
                        t = spool.tile([128, N], f32, tag=f"sf{g}")
                        # bench-only variant (wrong values, right timing)
                        nc.sync.dma_start(t[0:64, :], ag_in_h[0][:] if split_ag
                                          else ag_in[:])
                        nc.sync.dma_start(t[64:128, :], ag_in_h[1][:]
                                          if split_ag else ag_in[:])
                        sf.append(t)

                # ---- 8 waves: x = relu(x + x @ S), x column-major [128, 4]
                if xc is None:
                    xc = xpool.tile([128, 4], f32, tag="xc")
                    nc.sync.dma_start(xc[:], x0c[:])
                tail = None
                for _w in range(nwaves):
                    po = pwpool.tile([128, 4], f32, tag="po")
                    for g in range(4):        # output k-chunk
                        for ac in range(4):   # contraction a-chunk
                            nc.tensor.matmul(
                                po[:, g:g + 1],
                                lhsT=sf[ac][:, g * 128:(g + 1) * 128],
                                rhs=xc[:, ac:ac + 1],
                                start=(ac == 0),
                                stop=(ac == 3),
                            )
                    # x + offspring > 0 always in the fast path (x>0, S>=0),
                    # so the reference's relu is the identity here; skip it.
                    xn = xpool.tile([128, 4], f32, tag="xc")
                    if colwise_add:
                        # per-column adds so wave w+1's first matmuls can
                        # start as soon as their input column is ready
                        for g in range(4):
                            tail = nc.vector.tensor_add(
                                xn[:, g:g + 1], xc[:, g:g + 1], po[:, g:g + 1])
                    else:
                        tail = nc.vector.tensor_add(xn[:], xc[:], po[:])
                    xc = xn
                if nwaves == 0:
                    tail = nc.vector.tensor_copy(xc[:], sf[0][0:128, 0:4])
                prev_tail = tail
            nc.sync.dma_start(x_out[:], xc[:])

    nc.compile()
    return nc


# row permutation matching contig=True: qt row q*128+p must hold Qi col 4p+q
_QPERM = (4 * np.arange(128)[None, :] + np.arange(4)[:, None]).reshape(512)


def _in_maps(x_init: np.ndarray, Qi: np.ndarray, T: np.ndarray):
    x0c = np.ascontiguousarray(
        x_init.astype(np.float32).reshape(4, 128).T)  # x0c[p, g] = x[g*128+p]
    return [
        {
            "t_shard": T[c * SH:(c + 1) * SH],
            "q_cols": np.ascontiguousarray(
                Qi[c * SH:(c + 1) * SH].T[_QPERM]),
            "x0c": x0c,
        }
        for c in range(NC)
    ]


def get_program(reps: int = 1, mm: str = MM_DTYPE, **kw):
    key = (reps, mm, tuple(sorted(kw.items())))
    if key not in _prog_cache:
        _prog_cache[key] = _build_program(reps, mm, **kw)
    return _prog_cache[key]


def _run_device(x_init: np.ndarray, Qi: np.ndarray, T: np.ndarray) -> np.ndarray:
    # No NTFF hook exists in this chipless client; a stray BASS_TRACE=1
    # in the environment would crash run_bass_kernel_spmd otherwise.
    import os
    os.environ.setdefault("BASS_NEVER_TRACE", "1")
    import concourse.bass_utils as bass_utils
    global last_results

    nc = get_program()
    res = bass_utils.run_bass_kernel_spmd(
        nc, _in_maps(x_init, Qi, T), core_ids=list(range(NC)))
    last_results = res
    out = res.results[0]["x_out"]  # [128, 4]
    return np.ascontiguousarray(out.T).reshape(N).astype(np.float32)


def _reference_numpy(x0, logits, T, tgt):
    """Faithful per-wave fallback (any input values), pure numpy."""
    x = np.maximum(np.asarray(x0, dtype=np.float32), 0.0)
    logits = np.asarray(logits, dtype=np.float32)
    Tf = np.asarray(T, dtype=np.float32).reshape(N * N, N)
    for _ in range(NWAVES):
        p1 = x > 0.0
        p2 = p1.copy()
        p2[tgt] = False
        allowed = p1[:, None] & p2[None, :]
        masked = np.where(allowed, logits, np.float32(NEG_LARGE))
        m = masked.max(axis=1, keepdims=True)
        e = np.exp(masked - m, dtype=np.float32)
        probs = e / e.sum(axis=1, keepdims=True, dtype=np.float32)
        cnt = allowed.sum(axis=1, keepdims=True)
        Qi = np.where(cnt > 0, probs, np.float32(0.0)).astype(np.float32)
        w = (x[:, None] * Qi).reshape(N * N)
        offspring = w @ Tf
        x = np.maximum(x + offspring, 0.0).astype(np.float32)
    return x


def kernel(x0, logits, T, target_idx) -> np.ndarray:
    x0 = np.asarray(x0)
    logits = np.asarray(logits, dtype=np.float32)
    T = np.ascontiguousarray(np.asarray(T, dtype=np.float32))
    tgt = int(np.asarray(target_idx).ravel()[0])

    x_init = np.maximum(x0.astype(np.float32), 0.0)
    # Fast path requires the presence mask to be wave-invariant: guaranteed
    # when every x0 > 0 and T >= 0 (offspring >= 0 keeps x > 0 forever).
    if bool(np.all(x_init > 0.0)) and float(T.min()) >= 0.0:
        Qi = _qi_matrix(logits, tgt)
        try:
            return _run_device(x_init, Qi, T)
        except Exception:
            import traceback
            traceback.print_exc()
            print("kernel: device path failed; using numpy fallback")
    return _reference_numpy(x0, logits, T, tgt)



# revision 37
# speedup vs baseline: 1.1600x; 1.1600x over previous
"""Trainium2 Bass kernel for nn_BreedingPolicyNet (sparse_attention family).

Reference semantics (per wave, 8 waves):
    present_p1 = x > 0;  present_p2 = present_p1 with target_idx forced False
    allowed[a,b] = p1[a] & p2[b]
    Qi = softmax(where(allowed, logits, -FLT_MAX), axis=1), zeroed where row empty
    offspring[k] = sum_{a,b} x[a] * Qi[a,b] * T[a,b,k]
    x = max(x + offspring, 0)

Key algebraic property exploited: when every x0[i] > 0 and T >= 0, x stays
strictly positive through all waves (offspring >= 0), so the mask — and
therefore Qi — is IDENTICAL in every wave.  Then
    S[a,k] = sum_b Qi[a,b] * T[a,b,k]        (one single pass over T)
    offspring = x @ S                         (tiny per-wave matvec)
This turns 8 full 512MB passes over T into one (the memory roofline).

Distribution: shard T along axis a (contiguous 64MB per core).  Each core
computes its 64 rows of S with PE matmuls, an AllGather assembles the full
[512,512] S on every core, and all cores redundantly run the 8-wave
recurrence on-device.  Output is read from core 0.
"""

import numpy as np

N = 512
NC = 8           # NeuronCores
SH = N // NC     # a-rows per core
NWAVES = 8
AB = 8           # a-rows fetched per DMA (8MB chunks)
NEG_LARGE = float(np.finfo(np.float32).min)

_prog_cache = {}
last_results = None  # stash of BassKernelResults for test harness introspection


def _qi_matrix(logits: np.ndarray, tgt: int) -> np.ndarray:
    """Wave-invariant Qi: row softmax of logits with column `tgt` masked."""
    masked = np.array(logits, dtype=np.float32, copy=True)
    masked[:, tgt] = NEG_LARGE
    m = masked.max(axis=1, keepdims=True)
    e = np.exp(masked - m, dtype=np.float32)
    return (e / e.sum(axis=1, keepdims=True, dtype=np.float32)).astype(np.float32)


MM_DTYPE = "f32r"  # "f32" (exact, 4 cyc/row) or "f32r" (TF32-ish, 1 cyc/row)


def _build_program(reps: int = 1, mm: str = MM_DTYPE, nwaves: int = NWAVES,
                   ag: bool = True, ab: int = AB, tbufs: int = 2,
                   sbufs: int = 2, colwise_add: bool = True,
                   dual_ring: bool = False, split_ag: bool = True,
                   taper: bool = True, contig: bool = True,
                   stream_only: bool = False, store_eng: str = "sync",
                   wave_mode: str = "colT",
                   ag_segs: tuple = (32, 16, 8, 8),
                   sfload_eng: str | None = None,
                   stream_pp: int = 128):
    """Build + compile the SPMD program.

    reps > 1 emits the whole body N times, serialized end-to-start via an
    explicit dependency and chained through x — used only for benchmarking
    per-execution device time with dispatch overhead amortized out.

    mm selects the S-stage matmul dtype: float32r streams the moving
    operand at full rate (1 cycle/row vs 4 for float32), turning the
    S pass from PE-bound back into DMA-bound.  The wave matmuls always
    stay float32 — their cost is negligible and x spans 19 decades.
    """
    import concourse.bacc as bacc
    import concourse.bass as bass
    import concourse.mybir as mybir
    import concourse.tile as tile

    f32 = mybir.dt.float32
    fmm = mybir.dt.float32r if mm == "f32r" else f32
    nc = bacc.Bacc(
        "TRN2",
        target_bir_lowering=False,
        debug=False,
        enable_asserts=False,
        num_devices=NC,
    )
    # Wave-stage dtype is plain f32: f32r would stream S 4x faster, but no
    # compute engine can WRITE f32r-typed tiles (ISA restriction), and x is
    # rewritten by the DVE every wave.
    fxc = f32
    t_shard = nc.dram_tensor("t_shard", [SH, N, N], fmm, kind="ExternalInput").ap()
    q_cols = nc.dram_tensor("q_cols", [N, SH], fmm, kind="ExternalInput").ap()
    x0c = nc.dram_tensor("x0c", [128, 4], fxc, kind="ExternalInput").ap()
    x_out = nc.dram_tensor("x_out", [128, 4], f32, kind="ExternalOutput").ap()

    with tile.TileContext(nc) as tc:
        with (
            tc.tile_pool(name="const", bufs=1) as cpool,
            tc.tile_pool(name="tbuf", bufs=3) as tpool,
            tc.tile_pool(name="sfull", bufs=1) as spool,
            tc.tile_pool(name="xbuf", bufs=2) as xpool,
            tc.tile_pool(name="psum_s", bufs=4, space="PSUM") as pspool,
            tc.tile_pool(name="psum_w", bufs=2, space="PSUM") as pwpool,
            tc.tile_pool(name="psum_t", bufs=2, space="PSUM") as ptpool,
            tc.tile_pool(name="dram", bufs=1, space="DRAM") as dpool,
        ):
            # All non-T-stream DMAs go on `seng`'s queue so the big T loads
            # on nc.sync never wait behind a store that waits on compute.
            engs = {"sync": nc.sync, "scalar": nc.scalar,
                    "gpsimd": nc.gpsimd}
            seng = engs[store_eng]
            sleng = engs[sfload_eng or store_eng]
            ones = None
            if wave_mode == "colT":
                # [1,1] ones: rhs for the PE transposes in the wave stage
                ones = cpool.tile([1, 1], fxc, tag="ones")
                nc.vector.memset(ones[:], 1.0)
            # Qi columns for this core's a-shard: q_cols[b, a] = Qi[a0+a, b]
            qts = []
            for g in range(4):
                qt = cpool.tile([128, SH], fmm, tag=f"qt{g}")
                seng.dma_start(qt[:], q_cols[g * 128:(g + 1) * 128, :])
                qts.append(qt)

            xc = None
            prev_tail = None  # last instruction of previous rep (bench mode)
            if taper:
                sizes = [ab] * (SH // ab - 1) + [ab // 2, ab // 2]
            else:
                sizes = [ab] * (SH // ab)
            assert sum(sizes) == SH
            for rep in range(reps):
                if stream_only:
                    # T DMAs only — measures achievable stream bandwidth.
                    # stream_pp < 128 spreads each a-row over fewer
                    # partitions -> longer contiguous descriptors
                    # (pp=128: 8KB, 64: 16KB, 32: 32KB).
                    pp = stream_pp
                    hh = N // pp  # b-values per partition per row
                    ld = None
                    for ib, cs in enumerate(sizes):
                        tt = tpool.tile([pp, cs * hh * N], fmm, tag="tt",
                                        bufs=tbufs)
                        a0s = sum(sizes[:ib])
                        src = t_shard[a0s:a0s + cs]
                        eng = nc.scalar if (dual_ring and ib % 2) else nc.sync
                        ld = eng.dma_start(
                            tt[:].rearrange("p (j hk) -> p j hk", j=cs),
                            src.rearrange("j (p h) k -> p j (h k)",
                                          p=pp, h=hh),
                        )
                        if ib == 0 and prev_tail is not None:
                            bass._add_dep_helper(
                                ld.ins, prev_tail.ins, True,
                                "serialize bench rep")
                    prev_tail = ld
                    if xc is None:
                        xc = xpool.tile([128, 4], fxc, tag="xc")
                        nc.sync.dma_start(xc[:], x0c[:])
                    continue
                fsd = f32
                # segmented AllGather: each segment's gather is issued as
                # soon as its S rows are staged, hiding under the remaining
                # T stream; only the (small) last segment is a true tail.
                segs = list(ag_segs) if split_ag else [SH]
                assert sum(segs) == SH
                offs = [sum(segs[:i]) for i in range(len(segs) + 1)]
                ag_in_s = [dpool.tile([s, N], fsd, tag=f"ag_in{rep}s{i}",
                                      name=f"ag_in{rep}s{i}")
                           for i, s in enumerate(segs)]
                ag_out_s = [dpool.tile([NC * s, N], fsd,
                                       tag=f"ag_out{rep}s{i}",
                                       name=f"ag_out{rep}s{i}")
                            for i, s in enumerate(segs)]

                # ---- one pass over the T shard:
                # S[a,:] = sum_b Qi[a,b] * T[a,b,:].  S rows come out of the
                # PE as [1, 512] on partition 0; compute engines can only
                # write 32-aligned partition offsets, so stage SROWS of them
                # side-by-side in the free dim and DMA to DRAM.
                SROWS = 8
                a0 = 0
                for ib, cs in enumerate(sizes):
                    tt = tpool.tile([128, cs * 4 * N], fmm, tag="tt",
                                    bufs=tbufs)
                    src = t_shard[a0:a0 + cs]  # [cs, N, N]
                    eng = nc.scalar if (dual_ring and ib % 2) else nc.sync
                    if contig:
                        # partition p takes the contiguous 8KB span
                        # T[a, 4p:(4p+4), :] -> 4x longer DMA bursts; the
                        # b-contraction regroups as b = 4p+q and the Qi
                        # columns arrive pre-permuted to match (host-side).
                        ld = eng.dma_start(
                            tt[:].rearrange("p (j qk) -> p j qk", j=cs),
                            src.rearrange("j (p q) k -> p j (q k)",
                                          p=128, q=4),
                        )
                    else:
                        ld = eng.dma_start(
                            tt[:].rearrange("p (j g k) -> p j g k",
                                            j=cs, g=4),
                            src.rearrange("j (g p) k -> p j g k", g=4, p=128),
                        )
                    if ib == 0 and prev_tail is not None:
                        bass._add_dep_helper(
                            ld.ins, prev_tail.ins, True, "serialize bench rep")
                    for j in range(cs):
                        a = a0 + j
                        if a % SROWS == 0:
                            stage = cpool.tile([1, SROWS * N], f32,
                                               tag="stage", bufs=sbufs)
                        ps = pspool.tile([1, N], f32, tag="ps")
                        for g in range(4):
                            nc.tensor.matmul(
                                ps[:],
                                lhsT=qts[g][:, a:a + 1],
                                rhs=tt[:, (j * 4 + g) * N:(j * 4 + g + 1) * N],
                                start=(g == 0),
                                stop=(g == 3),
                            )
                        r = a % SROWS
                        nc.vector.tensor_copy(stage[:, r * N:(r + 1) * N],
                                              ps[:])
                        if r == SROWS - 1:
                            base = a - r
                            si = max(i for i in range(len(segs))
                                     if offs[i] <= base)
                            dst = ag_in_s[si][base - offs[si]:
                                              base - offs[si] + SROWS, :]
                            seng.dma_start(
                                dst.rearrange("(p r) k -> p r k", p=1),
                                stage[:].rearrange("p (r k) -> p r k",
                                                   r=SROWS),
                            )
                    a0 += cs

                # ---- AllGather the S shards into the full [512, 512] S
                sf = []
                if ag:
                    for i in range(len(segs)):
                        nc.gpsimd.collective_compute(
                            "AllGather",
                            mybir.AluOpType.bypass,
                            replica_groups=[list(range(NC))],
                            ins=[ag_in_s[i].opt()],
                            outs=[ag_out_s[i].opt()],
                        )
                    for g in range(4):
                        t = spool.tile([128, N], fsd, tag=f"sf{g}")
                        # tile g holds S rows [128g, 128g+128) = ranks 2g
                        # (partitions off..off+s) and 2g+1 (64+off..) per seg
                        for i, s in enumerate(segs):
                            off = offs[i]
                            for h in range(2):
                                sleng.dma_start(
                                    t[h * 64 + off:h * 64 + off + s, :],
                                    ag_out_s[i][(2 * g + h) * s:
                                                (2 * g + h + 1) * s, :])
                        sf.append(t)
                else:
                    assert segs[0] >= 32
                    for g in range(4):
                        t = spool.tile([128, N], fsd, tag=f"sf{g}")
                        # bench-only variant (wrong values, right timing):
                        # 4 × 32-row loads stand in for the post-AG sf fill
                        for h in range(4):
                            sleng.dma_start(
                                t[h * 32:(h + 1) * 32, :],
                                ag_in_s[0][0:32, :])
                        sf.append(t)

                # ---- 8 waves: x = relu(x + x @ S), x column-major [128, 4]
                if xc is None:
                    xc = xpool.tile([128, 4], fxc, tag="xc")
                    nc.sync.dma_start(xc[:], x0c[:])
                tail = None
                f32r = mybir.dt.float32r
                for _w in range(nwaves):
                    if wave_mode == "colT":
                        # x stationary: pk[1,k] = sum_ac x_chunk^T @ S_chunk,
                        # S streams at f32r (1 cyc/row).  Then 4 PE transposes
                        # turn the [1,512] offspring row back into x's
                        # column-major [128,4] for the next wave's weights.
                        pk = pwpool.tile([1, N], f32, tag="pk")
                        for ac in range(4):
                            nc.tensor.matmul(
                                pk[:],
                                lhsT=xc[:, ac:ac + 1],
                                rhs=sf[ac][:],
                                start=(ac == 0),
                                stop=(ac == 3),
                            )
                        pks = xpool.tile([1, N], fxc, tag="pks")
                        nc.vector.tensor_copy(pks[:], pk[:])
                        xn = xpool.tile([128, 4], fxc, tag="xc")
                        for g in range(4):
                            pt = ptpool.tile([128, 1], fxc, tag="pt")
                            nc.tensor.transpose(
                                pt[:], pks[:, g * 128:(g + 1) * 128], ones[:])
                            tail = nc.vector.tensor_add(
                                xn[:, g:g + 1], xc[:, g:g + 1], pt[:])
                        xc = xn
                        continue
                    po = pwpool.tile([128, 4], f32, tag="po")
                    for g in range(4):        # output k-chunk
                        for ac in range(4):   # contraction a-chunk
                            nc.tensor.matmul(
                                po[:, g:g + 1],
                                lhsT=sf[ac][:, g * 128:(g + 1) * 128],
                                rhs=xc[:, ac:ac + 1],
                                start=(ac == 0),
                                stop=(ac == 3),
                            )
                    # x + offspring > 0 always in the fast path (x>0, S>=0),
                    # so the reference's relu is the identity here; skip it.
                    xn = xpool.tile([128, 4], f32, tag="xc")
                    if colwise_add:
                        # per-column adds so wave w+1's first matmuls can
                        # start as soon as their input column is ready
                        for g in range(4):
                            tail = nc.vector.tensor_add(
                                xn[:, g:g + 1], xc[:, g:g + 1], po[:, g:g + 1])
                    else:
                        tail = nc.vector.tensor_add(xn[:], xc[:], po[:])
                    xc = xn
                if nwaves == 0:
                    tail = nc.vector.tensor_copy(
                        xc[:], sf[0][0:128, 0:4].bitcast(f32))
                prev_tail = tail
            nc.sync.dma_start(x_out[:], xc[:].bitcast(f32))

    nc.compile()
    return nc


# row permutation matching contig=True: qt row q*128+p must hold Qi col 4p+q
_QPERM = (4 * np.arange(128)[None, :] + np.arange(4)[:, None]).reshape(512)


def _in_maps(x_init: np.ndarray, Qi: np.ndarray, T: np.ndarray):
    x0c = np.ascontiguousarray(
        x_init.astype(np.float32).reshape(4, 128).T)  # x0c[p, g] = x[g*128+p]
    return [
        {
            "t_shard": T[c * SH:(c + 1) * SH],
            "q_cols": np.ascontiguousarray(
                Qi[c * SH:(c + 1) * SH].T[_QPERM]),
            "x0c": x0c,
        }
        for c in range(NC)
    ]


# Best-known config (sim-guided, HW-validated); bench variants override keys.
BEST = {
    "ag_segs": (32, 24, 8),
    "dual_ring": True,
    "store_eng": "scalar",
}


def get_program(reps: int = 1, mm: str = MM_DTYPE, **kw):
    cfg = dict(BEST)
    cfg.update(kw)
    key = (reps, mm, tuple(sorted(cfg.items())))
    if key not in _prog_cache:
        _prog_cache[key] = _build_program(reps, mm, **cfg)
    return _prog_cache[key]


def _run_device(x_init: np.ndarray, Qi: np.ndarray, T: np.ndarray) -> np.ndarray:
    # No NTFF hook exists in this chipless client; a stray BASS_TRACE=1
    # in the environment would crash run_bass_kernel_spmd otherwise.
    import os
    os.environ.setdefault("BASS_NEVER_TRACE", "1")
    import concourse.bass_utils as bass_utils
    global last_results

    nc = get_program()
    res = bass_utils.run_bass_kernel_spmd(
        nc, _in_maps(x_init, Qi, T), core_ids=list(range(NC)))
    last_results = res
    out = res.results[0]["x_out"]  # [128, 4]
    return np.ascontiguousarray(out.T).reshape(N).astype(np.float32)


def _reference_numpy(x0, logits, T, tgt):
    """Faithful per-wave fallback (any input values), pure numpy."""
    x = np.maximum(np.asarray(x0, dtype=np.float32), 0.0)
    logits = np.asarray(logits, dtype=np.float32)
    Tf = np.asarray(T, dtype=np.float32).reshape(N * N, N)
    for _ in range(NWAVES):
        p1 = x > 0.0
        p2 = p1.copy()
        p2[tgt] = False
        allowed = p1[:, None] & p2[None, :]
        masked = np.where(allowed, logits, np.float32(NEG_LARGE))
        m = masked.max(axis=1, keepdims=True)
        e = np.exp(masked - m, dtype=np.float32)
        probs = e / e.sum(axis=1, keepdims=True, dtype=np.float32)
        cnt = allowed.sum(axis=1, keepdims=True)
        Qi = np.where(cnt > 0, probs, np.float32(0.0)).astype(np.float32)
        w = (x[:, None] * Qi).reshape(N * N)
        offspring = w @ Tf
        x = np.maximum(x + offspring, 0.0).astype(np.float32)
    return x


def kernel(x0, logits, T, target_idx) -> np.ndarray:
    x0 = np.asarray(x0)
    logits = np.asarray(logits, dtype=np.float32)
    T = np.ascontiguousarray(np.asarray(T, dtype=np.float32))
    tgt = int(np.asarray(target_idx).ravel()[0])

    x_init = np.maximum(x0.astype(np.float32), 0.0)
    # Fast path requires the presence mask to be wave-invariant: guaranteed
    # when every x0 > 0 and T >= 0 (offspring >= 0 keeps x > 0 forever).
    if bool(np.all(x_init > 0.0)) and float(T.min()) >= 0.0:
        Qi = _qi_matrix(logits, tgt)
        try:
            return _run_device(x_init, Qi, T)
        except Exception:
            import traceback
            traceback.print_exc()
            print("kernel: device path failed; using numpy fallback")
    return _reference_numpy(x0, logits, T, tgt)



# revision 40
# speedup vs baseline: 1.2477x; 1.0756x over previous
"""Trainium2 Bass kernel for nn_BreedingPolicyNet (sparse_attention family).

Reference semantics (per wave, 8 waves):
    present_p1 = x > 0;  present_p2 = present_p1 with target_idx forced False
    allowed[a,b] = p1[a] & p2[b]
    Qi = softmax(where(allowed, logits, -FLT_MAX), axis=1), zeroed where row empty
    offspring[k] = sum_{a,b} x[a] * Qi[a,b] * T[a,b,k]
    x = max(x + offspring, 0)

Key algebraic property exploited: when every x0[i] > 0 and T >= 0, x stays
strictly positive through all waves (offspring >= 0), so the mask — and
therefore Qi — is IDENTICAL in every wave.  Then
    S[a,k] = sum_b Qi[a,b] * T[a,b,k]        (one single pass over T)
    offspring = x @ S                         (tiny per-wave matvec)
This turns 8 full 512MB passes over T into one (the memory roofline).

Distribution: shard T along axis a (contiguous 64MB per core).  Each core
computes its 64 rows of S with PE matmuls, an AllGather assembles the full
[512,512] S on every core, and all cores redundantly run the 8-wave
recurrence on-device.  Output is read from core 0.
"""

import numpy as np

N = 512
NC = 8           # NeuronCores
SH = N // NC     # a-rows per core
NWAVES = 8
AB = 8           # a-rows fetched per DMA (8MB chunks)
NEG_LARGE = float(np.finfo(np.float32).min)

_prog_cache = {}
last_results = None  # stash of BassKernelResults for test harness introspection


def _qi_matrix(logits: np.ndarray, tgt: int) -> np.ndarray:
    """Wave-invariant Qi: row softmax of logits with column `tgt` masked."""
    masked = np.array(logits, dtype=np.float32, copy=True)
    masked[:, tgt] = NEG_LARGE
    m = masked.max(axis=1, keepdims=True)
    e = np.exp(masked - m, dtype=np.float32)
    return (e / e.sum(axis=1, keepdims=True, dtype=np.float32)).astype(np.float32)


MM_DTYPE = "f32r"  # "f32" (exact, 4 cyc/row) or "f32r" (TF32-ish, 1 cyc/row)


def _build_program(reps: int = 1, mm: str = MM_DTYPE, nwaves: int = NWAVES,
                   ag: bool = True, ab: int = AB, tbufs: int = 2,
                   sbufs: int = 2, colwise_add: bool = True,
                   dual_ring: bool = False, split_ag: bool = True,
                   taper: bool = True, contig: bool = True,
                   stream_only: bool = False, store_eng: str = "sync",
                   wave_mode: str = "colT",
                   ag_segs: tuple = (32, 16, 8, 8),
                   sfload_eng: str | None = None,
                   stream_pp: int = 128, fused_add: bool = False):
    """Build + compile the SPMD program.

    reps > 1 emits the whole body N times, serialized end-to-start via an
    explicit dependency and chained through x — used only for benchmarking
    per-execution device time with dispatch overhead amortized out.

    mm selects the S-stage matmul dtype: float32r streams the moving
    operand at full rate (1 cycle/row vs 4 for float32), turning the
    S pass from PE-bound back into DMA-bound.  The wave matmuls always
    stay float32 — their cost is negligible and x spans 19 decades.
    """
    import concourse.bacc as bacc
    import concourse.bass as bass
    import concourse.mybir as mybir
    import concourse.tile as tile

    f32 = mybir.dt.float32
    fmm = mybir.dt.float32r if mm == "f32r" else f32
    nc = bacc.Bacc(
        "TRN2",
        target_bir_lowering=False,
        debug=False,
        enable_asserts=False,
        num_devices=NC,
    )
    # Wave-stage dtype is plain f32: f32r would stream S 4x faster, but no
    # compute engine can WRITE f32r-typed tiles (ISA restriction), and x is
    # rewritten by the DVE every wave.
    fxc = f32
    t_shard = nc.dram_tensor("t_shard", [SH, N, N], fmm, kind="ExternalInput").ap()
    q_cols = nc.dram_tensor("q_cols", [N, SH], fmm, kind="ExternalInput").ap()
    x0c = nc.dram_tensor("x0c", [128, 4], fxc, kind="ExternalInput").ap()
    x_out = nc.dram_tensor("x_out", [128, 4], f32, kind="ExternalOutput").ap()

    with tile.TileContext(nc) as tc:
        with (
            tc.tile_pool(name="const", bufs=1) as cpool,
            tc.tile_pool(name="tbuf", bufs=3) as tpool,
            tc.tile_pool(name="sfull", bufs=1) as spool,
            tc.tile_pool(name="xbuf", bufs=2) as xpool,
            tc.tile_pool(name="psum_s", bufs=4, space="PSUM") as pspool,
            tc.tile_pool(name="psum_w", bufs=2, space="PSUM") as pwpool,
            tc.tile_pool(name="psum_t", bufs=2, space="PSUM") as ptpool,
            tc.tile_pool(name="dram", bufs=1, space="DRAM") as dpool,
        ):
            # All non-T-stream DMAs go on `seng`'s queue so the big T loads
            # on nc.sync never wait behind a store that waits on compute.
            engs = {"sync": nc.sync, "scalar": nc.scalar,
                    "gpsimd": nc.gpsimd}
            seng = engs[store_eng]
            sleng = engs[sfload_eng or store_eng]
            ones = None
            if wave_mode == "colT":
                # [1,1] ones: rhs for the PE transposes in the wave stage
                ones = cpool.tile([1, 1], fxc, tag="ones")
                nc.vector.memset(ones[:], 1.0)
            # Qi columns for this core's a-shard: q_cols[b, a] = Qi[a0+a, b]
            qts = []
            for g in range(4):
                qt = cpool.tile([128, SH], fmm, tag=f"qt{g}")
                seng.dma_start(qt[:], q_cols[g * 128:(g + 1) * 128, :])
                qts.append(qt)

            xc = None
            prev_tail = None  # last instruction of previous rep (bench mode)
            if taper:
                sizes = [ab] * (SH // ab - 1) + [ab // 2, ab // 2]
            else:
                sizes = [ab] * (SH // ab)
            assert sum(sizes) == SH
            for rep in range(reps):
                if stream_only:
                    # T DMAs only — measures achievable stream bandwidth.
                    # stream_pp < 128 spreads each a-row over fewer
                    # partitions -> longer contiguous descriptors
                    # (pp=128: 8KB, 64: 16KB, 32: 32KB).
                    pp = stream_pp
                    hh = N // pp  # b-values per partition per row
                    ld = None
                    for ib, cs in enumerate(sizes):
                        tt = tpool.tile([pp, cs * hh * N], fmm, tag="tt",
                                        bufs=tbufs)
                        a0s = sum(sizes[:ib])
                        src = t_shard[a0s:a0s + cs]
                        eng = nc.scalar if (dual_ring and ib % 2) else nc.sync
                        ld = eng.dma_start(
                            tt[:].rearrange("p (j hk) -> p j hk", j=cs),
                            src.rearrange("j (p h) k -> p j (h k)",
                                          p=pp, h=hh),
                        )
                        if ib == 0 and prev_tail is not None:
                            bass._add_dep_helper(
                                ld.ins, prev_tail.ins, True,
                                "serialize bench rep")
                    prev_tail = ld
                    if xc is None:
                        xc = xpool.tile([128, 4], fxc, tag="xc")
                        nc.sync.dma_start(xc[:], x0c[:])
                    continue
                fsd = f32
                # segmented AllGather: each segment's gather is issued as
                # soon as its S rows are staged, hiding under the remaining
                # T stream; only the (small) last segment is a true tail.
                segs = list(ag_segs) if split_ag else [SH]
                assert sum(segs) == SH
                offs = [sum(segs[:i]) for i in range(len(segs) + 1)]
                ag_in_s = [dpool.tile([s, N], fsd, tag=f"ag_in{rep}s{i}",
                                      name=f"ag_in{rep}s{i}")
                           for i, s in enumerate(segs)]
                ag_out_s = [dpool.tile([NC * s, N], fsd,
                                       tag=f"ag_out{rep}s{i}",
                                       name=f"ag_out{rep}s{i}")
                            for i, s in enumerate(segs)]

                # ---- one pass over the T shard:
                # S[a,:] = sum_b Qi[a,b] * T[a,b,:].  S rows come out of the
                # PE as [1, 512] on partition 0; compute engines can only
                # write 32-aligned partition offsets, so stage SROWS of them
                # side-by-side in the free dim and DMA to DRAM.
                SROWS = 8
                a0 = 0
                for ib, cs in enumerate(sizes):
                    tt = tpool.tile([128, cs * 4 * N], fmm, tag="tt",
                                    bufs=tbufs)
                    src = t_shard[a0:a0 + cs]  # [cs, N, N]
                    eng = nc.scalar if (dual_ring and ib % 2) else nc.sync
                    if contig:
                        # partition p takes the contiguous 8KB span
                        # T[a, 4p:(4p+4), :] -> 4x longer DMA bursts; the
                        # b-contraction regroups as b = 4p+q and the Qi
                        # columns arrive pre-permuted to match (host-side).
                        ld = eng.dma_start(
                            tt[:].rearrange("p (j qk) -> p j qk", j=cs),
                            src.rearrange("j (p q) k -> p j (q k)",
                                          p=128, q=4),
                        )
                    else:
                        ld = eng.dma_start(
                            tt[:].rearrange("p (j g k) -> p j g k",
                                            j=cs, g=4),
                            src.rearrange("j (g p) k -> p j g k", g=4, p=128),
                        )
                    if ib == 0 and prev_tail is not None:
                        bass._add_dep_helper(
                            ld.ins, prev_tail.ins, True, "serialize bench rep")
                    for j in range(cs):
                        a = a0 + j
                        if a % SROWS == 0:
                            stage = cpool.tile([1, SROWS * N], f32,
                                               tag="stage", bufs=sbufs)
                        ps = pspool.tile([1, N], f32, tag="ps")
                        for g in range(4):
                            nc.tensor.matmul(
                                ps[:],
                                lhsT=qts[g][:, a:a + 1],
                                rhs=tt[:, (j * 4 + g) * N:(j * 4 + g + 1) * N],
                                start=(g == 0),
                                stop=(g == 3),
                            )
                        r = a % SROWS
                        nc.vector.tensor_copy(stage[:, r * N:(r + 1) * N],
                                              ps[:])
                        if r == SROWS - 1:
                            base = a - r
                            si = max(i for i in range(len(segs))
                                     if offs[i] <= base)
                            dst = ag_in_s[si][base - offs[si]:
                                              base - offs[si] + SROWS, :]
                            seng.dma_start(
                                dst.rearrange("(p r) k -> p r k", p=1),
                                stage[:].rearrange("p (r k) -> p r k",
                                                   r=SROWS),
                            )
                    a0 += cs

                # ---- AllGather the S shards into the full [512, 512] S
                sf = []
                if ag:
                    for i in range(len(segs)):
                        nc.gpsimd.collective_compute(
                            "AllGather",
                            mybir.AluOpType.bypass,
                            replica_groups=[list(range(NC))],
                            ins=[ag_in_s[i].opt()],
                            outs=[ag_out_s[i].opt()],
                        )
                    for g in range(4):
                        t = spool.tile([128, N], fsd, tag=f"sf{g}")
                        # tile g holds S rows [128g, 128g+128) = ranks 2g
                        # (partitions off..off+s) and 2g+1 (64+off..) per seg
                        for i, s in enumerate(segs):
                            off = offs[i]
                            for h in range(2):
                                sleng.dma_start(
                                    t[h * 64 + off:h * 64 + off + s, :],
                                    ag_out_s[i][(2 * g + h) * s:
                                                (2 * g + h + 1) * s, :])
                        sf.append(t)
                else:
                    assert segs[0] >= 32
                    for g in range(4):
                        t = spool.tile([128, N], fsd, tag=f"sf{g}")
                        # bench-only variant (wrong values, right timing):
                        # 4 × 32-row loads stand in for the post-AG sf fill
                        for h in range(4):
                            sleng.dma_start(
                                t[h * 32:(h + 1) * 32, :],
                                ag_in_s[0][0:32, :])
                        sf.append(t)

                # ---- 8 waves: x = relu(x + x @ S), x column-major [128, 4]
                if xc is None:
                    xc = xpool.tile([128, 4], fxc, tag="xc")
                    nc.sync.dma_start(xc[:], x0c[:])
                tail = None
                f32r = mybir.dt.float32r
                for _w in range(nwaves):
                    if wave_mode == "colT":
                        # x stationary: pk[1,k] = sum_ac x_chunk^T @ S_chunk,
                        # S streams at f32r (1 cyc/row).  Then 4 PE transposes
                        # turn the [1,512] offspring row back into x's
                        # column-major [128,4] for the next wave's weights.
                        pk = pwpool.tile([1, N], f32, tag="pk")
                        for ac in range(4):
                            nc.tensor.matmul(
                                pk[:],
                                lhsT=xc[:, ac:ac + 1],
                                rhs=sf[ac][:],
                                start=(ac == 0),
                                stop=(ac == 3),
                            )
                        pks = xpool.tile([1, N], fxc, tag="pks")
                        nc.vector.tensor_copy(pks[:], pk[:])
                        xn = xpool.tile([128, 4], fxc, tag="xc")
                        if fused_add:
                            # all 4 transposes land in one [128,4] PSUM tile;
                            # a single DVE add updates x (fewer instructions
                            # and semaphore hops per wave)
                            pt4 = ptpool.tile([128, 4], fxc, tag="pt")
                            for g in range(4):
                                nc.tensor.transpose(
                                    pt4[:, g:g + 1],
                                    pks[:, g * 128:(g + 1) * 128], ones[:])
                            tail = nc.vector.tensor_add(xn[:], xc[:], pt4[:])
                        else:
                            for g in range(4):
                                pt = ptpool.tile([128, 1], fxc, tag="pt")
                                nc.tensor.transpose(
                                    pt[:], pks[:, g * 128:(g + 1) * 128],
                                    ones[:])
                                tail = nc.vector.tensor_add(
                                    xn[:, g:g + 1], xc[:, g:g + 1], pt[:])
                        xc = xn
                        continue
                    po = pwpool.tile([128, 4], f32, tag="po")
                    for g in range(4):        # output k-chunk
                        for ac in range(4):   # contraction a-chunk
                            nc.tensor.matmul(
                                po[:, g:g + 1],
                                lhsT=sf[ac][:, g * 128:(g + 1) * 128],
                                rhs=xc[:, ac:ac + 1],
                                start=(ac == 0),
                                stop=(ac == 3),
                            )
                    # x + offspring > 0 always in the fast path (x>0, S>=0),
                    # so the reference's relu is the identity here; skip it.
                    xn = xpool.tile([128, 4], f32, tag="xc")
                    if colwise_add:
                        # per-column adds so wave w+1's first matmuls can
                        # start as soon as their input column is ready
                        for g in range(4):
                            tail = nc.vector.tensor_add(
                                xn[:, g:g + 1], xc[:, g:g + 1], po[:, g:g + 1])
                    else:
                        tail = nc.vector.tensor_add(xn[:], xc[:], po[:])
                    xc = xn
                if nwaves == 0:
                    tail = nc.vector.tensor_copy(
                        xc[:], sf[0][0:128, 0:4].bitcast(f32))
                prev_tail = tail
            nc.sync.dma_start(x_out[:], xc[:].bitcast(f32))

    nc.compile()
    return nc


# row permutation matching contig=True: qt row q*128+p must hold Qi col 4p+q
_QPERM = (4 * np.arange(128)[None, :] + np.arange(4)[:, None]).reshape(512)


def _in_maps(x_init: np.ndarray, Qi: np.ndarray, T: np.ndarray):
    x0c = np.ascontiguousarray(
        x_init.astype(np.float32).reshape(4, 128).T)  # x0c[p, g] = x[g*128+p]
    return [
        {
            "t_shard": T[c * SH:(c + 1) * SH],
            "q_cols": np.ascontiguousarray(
                Qi[c * SH:(c + 1) * SH].T[_QPERM]),
            "x0c": x0c,
        }
        for c in range(NC)
    ]


# Best-known config (sim-guided, HW-validated); bench variants override keys.
BEST = {
    "ag_segs": (32, 24, 8),
    "dual_ring": True,
    "store_eng": "scalar",
    "fused_add": True,
}


def get_program(reps: int = 1, mm: str = MM_DTYPE, **kw):
    cfg = dict(BEST)
    cfg.update(kw)
    key = (reps, mm, tuple(sorted(cfg.items())))
    if key not in _prog_cache:
        _prog_cache[key] = _build_program(reps, mm, **cfg)
    return _prog_cache[key]


def _run_device(x_init: np.ndarray, Qi: np.ndarray, T: np.ndarray) -> np.ndarray:
    # No NTFF hook exists in this chipless client; a stray BASS_TRACE=1
    # in the environment would crash run_bass_kernel_spmd otherwise.
    import os
    os.environ.setdefault("BASS_NEVER_TRACE", "1")
    import concourse.bass_utils as bass_utils
    global last_results

    nc = get_program()
    res = bass_utils.run_bass_kernel_spmd(
        nc, _in_maps(x_init, Qi, T), core_ids=list(range(NC)))
    last_results = res
    out = res.results[0]["x_out"]  # [128, 4]
    return np.ascontiguousarray(out.T).reshape(N).astype(np.float32)


def _reference_numpy(x0, logits, T, tgt):
    """Faithful per-wave fallback (any input values), pure numpy."""
    x = np.maximum(np.asarray(x0, dtype=np.float32), 0.0)
    logits = np.asarray(logits, dtype=np.float32)
    Tf = np.asarray(T, dtype=np.float32).reshape(N * N, N)
    for _ in range(NWAVES):
        p1 = x > 0.0
        p2 = p1.copy()
        p2[tgt] = False
        allowed = p1[:, None] & p2[None, :]
        masked = np.where(allowed, logits, np.float32(NEG_LARGE))
        m = masked.max(axis=1, keepdims=True)
        e = np.exp(masked - m, dtype=np.float32)
        probs = e / e.sum(axis=1, keepdims=True, dtype=np.float32)
        cnt = allowed.sum(axis=1, keepdims=True)
        Qi = np.where(cnt > 0, probs, np.float32(0.0)).astype(np.float32)
        w = (x[:, None] * Qi).reshape(N * N)
        offspring = w @ Tf
        x = np.maximum(x + offspring, 0.0).astype(np.float32)
    return x


def kernel(x0, logits, T, target_idx) -> np.ndarray:
    x0 = np.asarray(x0)
    logits = np.asarray(logits, dtype=np.float32)
    T = np.ascontiguousarray(np.asarray(T, dtype=np.float32))
    tgt = int(np.asarray(target_idx).ravel()[0])

    x_init = np.maximum(x0.astype(np.float32), 0.0)
    # Fast path requires the presence mask to be wave-invariant: guaranteed
    # when every x0 > 0 and T >= 0 (offspring >= 0 keeps x > 0 forever).
    if bool(np.all(x_init > 0.0)) and float(T.min()) >= 0.0:
        Qi = _qi_matrix(logits, tgt)
        try:
            return _run_device(x_init, Qi, T)
        except Exception:
            import traceback
            traceback.print_exc()
            print("kernel: device path failed; using numpy fallback")
    return _reference_numpy(x0, logits, T, tgt)

